# revision 1
# baseline (speedup 1.0000x reference)
"""Grouped-experts SwiGLU MoE kernel for 8 Trainium2 NeuronCores.

Problem: x[16384, 2048] routed to 64 experts (256 contiguous tokens each);
per expert e: out_e = (silu(x_e @ w1[e]) * (x_e @ w3[e])) @ w2[e].

Sharding: expert-parallel. Core c owns experts 8c..8c+7 and therefore tokens
[2048c, 2048(c+1)) — tokens are pre-permuted so no all-to-all is needed: each
core computes its own token slice fully locally.

Per-core device program (memory-bound: streams 192 MiB of weights/core):
  per expert (8 per core):
    1. PE-transpose x_e [256, 2048] -> xT [dim, tok] tiles in SBUF
    2. gu accumulation (weights stationary, xT moving, fp32r matmuls):
       g^T[m] / u^T[m] [128 hid, 256 tok] accumulate over 16 k-tiles in PSUM
    3. h^T = silu(g^T) * u^T  (ACT silu + DVE mul) -> SBUF, fp32r
    4. out = h @ w2 via (hT stationary, w2 moving): out[tok, dim] in natural
       layout, accumulated over 8 hidden k-tiles, n-major to bound PSUM use.
Weights are host-repacked so every DMA is a contiguous 1-2 MiB transfer.
All matmuls use float32r (full PE rate at moving dim >= 256, ~1e-4 rel err).
"""

import numpy as np

import concourse.bacc as bacc
import concourse.mybir as mybir
from concourse.bass_utils import run_bass_kernel_spmd
from concourse.masks import make_identity
from concourse.tile import TileContext

N_CORES = 8
E_PER_CORE = 8          # experts per core
TOK_PER_E = 256         # tokens per expert
DIM = 2048
HID = 1024
P = 128
KT = DIM // P           # 16 k-tiles (contraction over dim)
KT2 = HID // P          # 8 k-tiles (contraction over hidden)
MT = HID // P           # 8 hidden m-tiles in stage 1
NCH = DIM // 512        # 4 output n-chunks of 512 in stage 2

F32 = mybir.dt.float32
F32R = mybir.dt.float32r
SILU = mybir.ActivationFunctionType.Silu
MULT = mybir.AluOpType.mult

_program_cache = {}


def _build_program():
    """Per-core Bass program. Same program for all 8 cores (SPMD)."""
    nc = bacc.Bacc("TRN2", target_bir_lowering=False, debug=False)

    x_d = nc.dram_tensor("x", [E_PER_CORE * TOK_PER_E, DIM], F32,
                         kind="ExternalInput")
    # w13: per (e, k-tile): [128, 2048] = [w1 k-block | w3 k-block], contiguous
    w13_d = nc.dram_tensor("w13", [E_PER_CORE * KT * P, 2 * HID], F32R,
                           kind="ExternalInput")
    # w2p: per (e, n, k2): [128, 512] contiguous; (e, n) group = 2 MiB
    w2_d = nc.dram_tensor("w2p", [E_PER_CORE * NCH * KT2 * P, 512], F32R,
                          kind="ExternalInput")
    out_d = nc.dram_tensor("out", [E_PER_CORE * TOK_PER_E, DIM], F32,
                           kind="ExternalOutput")

    with TileContext(nc) as tc:
        with tc.tile_pool(name="const", bufs=1) as constp, \
             tc.tile_pool(name="xn", bufs=2) as xn_p, \
             tc.tile_pool(name="xT", bufs=20) as xT_p, \
             tc.tile_pool(name="w13", bufs=3) as w13_p, \
             tc.tile_pool(name="w2", bufs=2) as w2_p, \
             tc.tile_pool(name="hT", bufs=12) as hT_p, \
             tc.tile_pool(name="gs", bufs=3) as gs_p, \
             tc.tile_pool(name="osb", bufs=3) as osb_p, \
             tc.tile_pool(name="ps", bufs=8, space="PSUM") as ps_p:

            ident = constp.tile([P, P], F32)
            make_identity(nc, ident)
            zbias = constp.tile([P, 1], F32)
            nc.vector.memset(zbias, 0.0)

            for e in range(E_PER_CORE):
                tok0 = e * TOK_PER_E

                # ---- load x_e and transpose to xT[k] [128 d, 256 t] ----
                xn = xn_p.tile([P, 2, DIM], F32, tag="xn")
                nc.scalar.dma_start(
                    out=xn[:],
                    in_=x_d[tok0:tok0 + TOK_PER_E, :].rearrange(
                        "(m p) c -> p m c", p=P),
                )
                xT = []
                for k in range(KT):
                    xt = xT_p.tile([P, TOK_PER_E], F32R, tag="xT")
                    xT.append(xt)
                    for mt in range(2):
                        tp = ps_p.tile([P, P], F32, tag="ps")
                        nc.tensor.transpose(
                            tp[:], xn[:, mt, k * P:(k + 1) * P], ident[:])
                        nc.vector.tensor_copy(
                            xt[:, mt * P:(mt + 1) * P], tp[:])

                # ---- stage 1: g/u accumulation over dim ----
                gu = [ps_p.tile([P, 512], F32, tag="ps", name=f"gu_e{e}_m{m}")
                      for m in range(MT)]
                for kk in range(KT // 2):
                    wt = w13_p.tile([P, 2, 2 * HID], F32R, tag="w13")
                    row0 = (e * KT + 2 * kk) * P
                    nc.sync.dma_start(
                        out=wt[:],
                        in_=w13_d[row0:row0 + 2 * P, :].rearrange(
                            "(h p) c -> p h c", p=P),
                    )
                    for half in range(2):
                        k = 2 * kk + half
                        # start=True clears has_written for the WHOLE bank, so
                        # only the first matmul into each gu bank may set it;
                        # the first w3 matmul overwrites via has_written=0.
                        for m in range(MT):
                            nc.tensor.matmul(
                                gu[m][:, 0:256],
                                lhsT=wt[:, half, m * P:(m + 1) * P],
                                rhs=xT[k][:], start=(k == 0),
                                stop=(k == KT - 1), skip_group_check=True)
                            nc.tensor.matmul(
                                gu[m][:, 256:512],
                                lhsT=wt[:, half, HID + m * P:HID + (m + 1) * P],
                                rhs=xT[k][:], start=False,
                                stop=(k == KT - 1), skip_group_check=True)

                # ---- h^T = silu(g^T) * u^T ----
                hT = []
                for m in range(MT):
                    gs = gs_p.tile([P, 256], F32, tag="gs")
                    nc.scalar.activation(gs[:], gu[m][:, 0:256], SILU,
                                         bias=zbias[:])
                    ht = hT_p.tile([P, 256], F32R, tag="hT")
                    hT.append(ht)
                    nc.vector.tensor_tensor(ht[:], gs[:], gu[m][:, 256:512],
                                            MULT)

                # ---- stage 2: out = h @ w2, n-major ----
                osb = osb_p.tile([P, 2, DIM], F32, tag="osb")
                for n in range(NCH):
                    w2t = w2_p.tile([P, KT2, 512], F32R, tag="w2")
                    row0 = (e * NCH + n) * KT2 * P
                    nc.sync.dma_start(
                        out=w2t[:],
                        in_=w2_d[row0:row0 + KT2 * P, :].rearrange(
                            "(k p) c -> p k c", p=P),
                    )
                    for m2 in range(2):
                        ops = ps_p.tile([P, 512], F32, tag="ps")
                        for k2 in range(KT2):
                            nc.tensor.matmul(
                                ops[:],
                                lhsT=hT[k2][:, m2 * P:(m2 + 1) * P],
                                rhs=w2t[:, k2, :],
                                start=(k2 == 0), stop=(k2 == KT2 - 1))
                        nc.vector.tensor_copy(
                            osb[:, m2, n * 512:(n + 1) * 512], ops[:])

                nc.scalar.dma_start(
                    out=out_d[tok0:tok0 + TOK_PER_E, :].rearrange(
                        "(m p) c -> p m c", p=P),
                    in_=osb[:],
                )

    nc.compile()
    return nc


def _get_program():
    if "nc" not in _program_cache:
        _program_cache["nc"] = _build_program()
    return _program_cache["nc"]


def kernel(x, w1, w2, w3, num_local_tokens_per_expert=None, **_unused):
    x = np.ascontiguousarray(np.asarray(x, dtype=np.float32))
    w1 = np.asarray(w1, dtype=np.float32)
    w2 = np.asarray(w2, dtype=np.float32)
    w3 = np.asarray(w3, dtype=np.float32)

    E = w1.shape[0]
    assert E == N_CORES * E_PER_CORE and x.shape == (E * TOK_PER_E, DIM)

    # host repack: w13[e, k, p, :] = [w1[e, kP+p, :] | w3[e, kP+p, :]]
    w13 = np.concatenate(
        [w1.reshape(E, KT, P, HID), w3.reshape(E, KT, P, HID)], axis=3)
    # w2p[e, n, k2, p, :] = w2[e, k2*P + p, n*512:(n+1)*512]
    w2p = np.ascontiguousarray(
        w2.reshape(E, KT2, P, NCH, 512).transpose(0, 3, 1, 2, 4))

    in_maps = []
    for c in range(N_CORES):
        e0 = c * E_PER_CORE
        in_maps.append({
            "x": x[c * E_PER_CORE * TOK_PER_E:(c + 1) * E_PER_CORE * TOK_PER_E],
            "w13": w13[e0:e0 + E_PER_CORE].reshape(E_PER_CORE * KT * P,
                                                   2 * HID),
            "w2p": w2p[e0:e0 + E_PER_CORE].reshape(E_PER_CORE * NCH * KT2 * P,
                                                   512),
        })

    nc = _get_program()
    res = run_bass_kernel_spmd(nc, in_maps, list(range(N_CORES)))
    return np.concatenate([res.results[c]["out"] for c in range(N_CORES)],
                          axis=0)

